# revision 4
# baseline (speedup 1.0000x reference)
"""Distributed attention kernel for one TRN2 chip (8 NeuronCores), v2.

Sharding: 16 heads / 8 cores = 2 heads per core (head-group parallel).

The v2 schedule is built around three hardware facts:
  - the PE clock is HAM-gated: ~1.2 GHz until ~3.4us of continuous matmul
    activity, 2.4 GHz after - so the kernel front-loads dummy matmuls to
    warm the array while x streams in, and keeps the PE queue non-empty
    throughout;
  - DMA descriptor generation serializes on the two HWDGE rings - so x and
    the weights are pre-laid-out on the host such that every DMA is 128
    contiguous 8KB (x granule halves) / 2KB (weights) descriptors, with x
    arriving in 512-token granules split across both rings;
  - the ACT (scalar) sequencer issues nothing but EXP once attention
    starts - all steady-state DMAs (allgather in/out, output writes) sit on
    the sync ring.

Dataflow per core: QKV for its 2 heads; attention per (batch, head,
512-query chunk) with scores in [k, q] layout (P^T feeds the PV matmul
directly), exp on ACT (scores bounded - no max subtraction), softmax
denominator via a ones column in V. The 16 attention units form a single
software pipeline: each unit emits the NEXT unit's first two score
matmuls in its last two groups, so the ACT engine never waits at unit or
allgather boundaries. Remaining QKV chunks and the output projections are
an ordered task stream injected one per group. AllGather of bf16
c-slices per (batch, qc) runs under later units; projection against the
core's 128-column w_proj slice is pipelined behind its allgather. Host
concatenates the 8 column slices.
"""

import numpy as np

_CACHE = {}

P = 128
B, T, C = 2, 2048, 1024
BT = B * T
NCORE = 8
HD = 64  # head dim
CSL = 128  # per-core c-slice = 2 heads * 64
TQ = 512  # query chunk
NQC = T // TQ  # 4
KC = 128  # key chunk (partition dim)
NKC = T // KC  # 16
KG = 2  # key chunks per exp group
NG = NKC // KG  # 8
NCC = C // P  # 8 contraction chunks
NTC = BT // P  # 32 token chunks of 128
TB = T // P  # 16 token chunks per batch
NGR = 8  # x arrival granules (512 tokens each)
GRL = BT // NGR  # 512


def _build():
    import concourse.bass as bass
    import concourse.tile as tile
    from concourse import bacc, mybir

    F32 = mybir.dt.float32
    BF16 = mybir.dt.bfloat16
    Exp = mybir.ActivationFunctionType.Exp

    nc = bacc.Bacc("TRN2", target_bir_lowering=False, debug=False, num_devices=NCORE)

    # host-prepared layouts (see _shard_inputs):
    #   x:  [P, NGR, NCC, GRL]  (c-chunk-major, 512-token granules)
    #   w*: [P, NCC, CSL]
    x_ext = nc.declare_dram_parameter("x", [P, NGR, NCC, GRL], BF16, isOutput=False)
    wq_ext = nc.declare_dram_parameter("wq", [P, NCC, CSL], BF16, isOutput=False)
    wk_ext = nc.declare_dram_parameter("wk", [P, NCC, CSL], BF16, isOutput=False)
    wv_ext = nc.declare_dram_parameter("wv", [P, NCC, CSL], BF16, isOutput=False)
    wp_ext = nc.declare_dram_parameter("wp", [P, NCC, CSL], BF16, isOutput=False)
    bq_ext = nc.declare_dram_parameter("bq", [CSL, 1], F32, isOutput=False)
    bk_ext = nc.declare_dram_parameter("bk", [CSL, 1], F32, isOutput=False)
    bv_ext = nc.declare_dram_parameter("bv", [1, CSL], F32, isOutput=False)
    bp_ext = nc.declare_dram_parameter("bp", [CSL, 1], F32, isOutput=False)
    # transposed output [CSL, BT]; the host transposes back
    out_ext = nc.declare_dram_parameter("out", [CSL, BT], F32, isOutput=True)

    rg = [list(range(NCORE))]

    with tile.TileContext(nc) as tc:
        with (
            nc.allow_low_precision("bf16 attention compute by design"),
            tc.tile_pool(name="pers", bufs=1) as pers,
            tc.tile_pool(name="stage", bufs=3) as stage,
            tc.tile_pool(name="dram", bufs=1, space="DRAM") as dram,
        ):
            # ---- persistent SBUF tiles ----
            xt_sb = pers.tile([P, NGR, NCC, GRL], BF16, name="xt_sb")
            qt_sb = pers.tile([P, BT], BF16, name="qt_sb")  # Q^T (rows: 2*64 head dims)
            kt_sb = pers.tile([P, BT], BF16, name="kt_sb")
            # per-head score operands with the head's 64 dims at partitions
            # 0-63 and zeros above: K=128 stationary tiles keep fast weight
            # load enabled for the score matmuls
            ktp_sb = [pers.tile([P, BT], BF16, name=f"ktp{h}_sb") for h in range(2)]
            qtp_sb = pers.tile([P, BT], BF16, name="qtp_sb")  # h1 only
            v_sb = pers.tile([P, NTC, 256], BF16, name="v_sb")  # V + ones col, 128/head
            ot_sb = pers.tile([P, BT], BF16, name="ot_sb")  # attention out^T
            wq_sb = pers.tile([P, NCC, CSL], BF16, name="wq_sb")
            wk_sb = pers.tile([P, NCC, CSL], BF16, name="wk_sb")
            wv_sb = pers.tile([P, NCC, CSL], BF16, name="wv_sb")
            wp_sb = pers.tile([P, NCC, CSL], BF16, name="wp_sb")
            bq_sb = pers.tile([CSL, 1], F32, name="bq_sb")
            bk_sb = pers.tile([CSL, 1], F32, name="bk_sb")
            bv_row = pers.tile([1, CSL], F32, name="bv_row")
            bp_sb = pers.tile([CSL, 1], F32, name="bp_sb")
            bv_bc = pers.tile([P, CSL], F32, name="bv_bc")
            ones1 = pers.tile([1, P], F32, name="ones1")
            warm_sb = pers.tile([P, 640], BF16, name="warm_sb")
            act_seed = pers.tile([1, 2], F32, name="act_seed")

            nc.gpsimd.memset(ones1[:], 1.0)
            nc.gpsimd.memset(warm_sb[:], 0.0)
            nc.gpsimd.memset(act_seed[:], 0.0)
            barrier_cc = []  # emitted onto the gpsimd queue after bar_in's DMA
            nc.gpsimd.memset(v_sb[:, :, 65:128], 0.0)
            nc.gpsimd.memset(v_sb[:, :, 193:256], 0.0)
            nc.gpsimd.memset(v_sb[:, :, 64], 1.0)
            nc.gpsimd.memset(v_sb[:, :, 192], 1.0)
            nc.gpsimd.memset(ktp_sb[0][HD:P, :], 0.0)
            nc.gpsimd.memset(ktp_sb[1][HD:P, :], 0.0)
            nc.gpsimd.memset(qtp_sb[:], 0.0)

            # ---- DMA issue: fast small stuff first, then x granules with
            # halves split across the two HWDGE rings so granule i lands
            # strictly before granule i+1.
            nc.sync.dma_start(bq_sb[:], bq_ext[:])
            nc.sync.dma_start(bk_sb[:], bk_ext[:])
            nc.sync.dma_start(bv_row[:], bv_ext[:])
            nc.sync.dma_start(bp_sb[:], bp_ext[:])
            for ext, dst in ((wk_ext, wk_sb), (wv_ext, wv_sb), (wq_ext, wq_sb), (wp_ext, wp_sb)):
                nc.scalar.dma_start(dst[:], ext[:])
            for g in range(NGR):
                nc.sync.dma_start(xt_sb[:, g, 0:4, :], x_ext[:, g, 0:4, :])
                nc.scalar.dma_start(xt_sb[:, g, 4:8, :], x_ext[:, g, 4:8, :])

            # preload the ACT Exp table while DMA streams
            nc.scalar.activation(act_seed[0:1, 0:1], act_seed[0:1, 1:2], Exp)

            # tiny barrier collective at t0: absorbs the cross-core start
            # skew while the PE warms up, so the first real allgather's
            # barrier doesn't eat it
            bar_in = dram.tile([1, 2], F32, tag="bar_in", name="bar_in")
            bar_out = dram.tile([NCORE, 2], F32, addr_space="Shared",
                                tag="bar_out", name="bar_out")
            nc.sync.dma_start(bar_in[:], act_seed[:])
            nc.gpsimd.collective_compute(
                "AllGather",
                mybir.AluOpType.bypass,
                ins=[bar_in.opt()],
                outs=[bar_out.opt()],
                replica_groups=rg,
            )

            def xt(c, t0, n):
                g, off = divmod(t0, GRL)
                assert off + n <= GRL
                return xt_sb[:, g, c, off:off + n]

            ag_in = {}
            ag_out = {}
            for b in range(B):
                for qc in range(NQC):
                    ag_in[(b, qc)] = dram.tile(
                        [CSL, TQ], BF16, tag=f"agin_{b}_{qc}", name=f"agin_{b}_{qc}")
                    ag_out[(b, qc)] = dram.tile(
                        [NCORE * CSL, TQ], BF16, addr_space="Shared",
                        tag=f"agout_{b}_{qc}", name=f"agout_{b}_{qc}")

            with (
                tc.tile_pool(name="psB", bufs=1, space="PSUM") as psB,
                tc.tile_pool(name="ptp", bufs=3) as ptp,
            ):
                def mm_ps():
                    return psB.tile([P, TQ], F32, tag="mm", bufs=2, name="ps_mm")

                def qkv_qk_chunk(w_sb, b_sb, dst, t8):
                    ps = mm_ps()
                    rng = slice(t8 * TQ, (t8 + 1) * TQ)
                    for c in range(NCC):
                        nc.tensor.matmul(
                            ps[:], w_sb[:, c, :], xt(c, t8 * TQ, TQ),
                            start=(c == 0), stop=(c == NCC - 1),
                        )
                    if dst is kt_sb:
                        # h0 rows straight into the padded tile; h1 rows
                        # staged then partition-shifted down via DMA
                        nc.vector.tensor_scalar_add(
                            ktp_sb[0][0:HD, rng], ps[0:HD, :], b_sb[0:HD, :])
                        nc.vector.tensor_scalar_add(
                            kt_sb[HD:P, rng], ps[HD:P, :], b_sb[HD:P, :])
                        nc.sync.dma_start(ktp_sb[1][0:HD, rng], kt_sb[HD:P, rng])
                    else:
                        nc.vector.tensor_scalar_add(qt_sb[:, rng], ps[:], b_sb[:])
                        nc.sync.dma_start(qtp_sb[0:HD, rng], qt_sb[HD:P, rng])

                def qkv_v_pair(i):
                    # two 128-token V chunks (half an x granule)
                    for j in (i, i + 1):
                        ps = mm_ps()
                        for c in range(NCC):
                            nc.tensor.matmul(
                                ps[:, 0:CSL], xt(c, j * P, P), wv_sb[:, c, :],
                                start=(c == 0), stop=(c == NCC - 1),
                            )
                        nc.vector.tensor_add(v_sb[:, j, 0:HD], ps[:, 0:HD], bv_bc[:, 0:HD])
                        nc.vector.tensor_add(
                            v_sb[:, j, 128:128 + HD], ps[:, HD:2 * HD], bv_bc[:, HD:2 * HD])

                def warmup(n):
                    # dead-write matmuls that flip the PE HAM to full clock
                    # while x streams in
                    for _ in range(n):
                        ps = mm_ps()
                        nc.tensor.matmul(
                            ps[:], warm_sb[:, 0:128], warm_sb[:, 128:640],
                            start=True, stop=True)

                # broadcast the V free-axis bias across partitions (K=1 matmul)
                bb = mm_ps()
                nc.tensor.matmul(bb[:, 0:CSL], ones1[0:1, :], bv_row[:], start=True, stop=True)
                nc.vector.tensor_copy(bv_bc[:], bb[:, 0:CSL])

                warmup(7)

                # minimal pre-attention QKV: K t8=0,1 / V chunks 0-3 / Q t8=0
                qkv_qk_chunk(wk_sb, bk_sb, kt_sb, 0)
                qkv_qk_chunk(wk_sb, bk_sb, kt_sb, 1)
                qkv_v_pair(0)
                qkv_v_pair(2)
                qkv_qk_chunk(wq_sb, bq_sb, qt_sb, 0)

                # remaining QKV as an ordered task stream injected one per
                # group; order tracks x-granule arrival and first-use
                # deadlines inside unit (0,0,h0).
                tasks = [
                    lambda: qkv_qk_chunk(wk_sb, bk_sb, kt_sb, 2),
                    lambda: qkv_v_pair(4),
                    lambda: qkv_v_pair(6),
                    lambda: qkv_qk_chunk(wk_sb, bk_sb, kt_sb, 3),
                    lambda: qkv_v_pair(8),
                    lambda: qkv_v_pair(10),
                    lambda: qkv_v_pair(12),
                    lambda: qkv_v_pair(14),
                    lambda: qkv_qk_chunk(wq_sb, bq_sb, qt_sb, 1),
                    lambda: qkv_qk_chunk(wk_sb, bk_sb, kt_sb, 4),
                    lambda: qkv_qk_chunk(wk_sb, bk_sb, kt_sb, 5),
                    lambda: qkv_qk_chunk(wq_sb, bq_sb, qt_sb, 2),
                    lambda: qkv_qk_chunk(wk_sb, bk_sb, kt_sb, 6),
                    lambda: qkv_qk_chunk(wk_sb, bk_sb, kt_sb, 7),
                    lambda: qkv_qk_chunk(wq_sb, bq_sb, qt_sb, 3),
                    lambda: qkv_v_pair(16),
                    lambda: qkv_v_pair(18),
                    lambda: qkv_v_pair(20),
                    lambda: qkv_v_pair(22),
                    lambda: qkv_v_pair(24),
                    lambda: qkv_v_pair(26),
                    lambda: qkv_v_pair(28),
                    lambda: qkv_v_pair(30),
                    lambda: qkv_qk_chunk(wq_sb, bq_sb, qt_sb, 4),
                    lambda: qkv_qk_chunk(wq_sb, bq_sb, qt_sb, 5),
                    lambda: qkv_qk_chunk(wq_sb, bq_sb, qt_sb, 6),
                    lambda: qkv_qk_chunk(wq_sb, bq_sb, qt_sb, 7),
                ]

                pend_a = []  # drain part A: denom copy + reciprocal + bcast
                pend_b = []  # drain part B: DVE normalize-multiply

                def flush(which):
                    while which:
                        which.pop(0)()

                def flush_all():
                    flush(pend_a)
                    flush(pend_b)

                # ---- attention unit pipeline ----
                units = [(b, qc, hh) for b in range(B) for qc in range(NQC)
                         for hh in range(2)]
                state = {}

                def unit_state(u):
                    if u not in state:
                        state[u] = {
                            "op_t": psB.tile([P, TQ], F32, tag="acc", bufs=2, name="op_t"),
                            "sps": {},
                        }
                    return state[u]

                def do_S(u, g):
                    b, qc, hh = u
                    st = unit_state(u)
                    base = b * T + qc * TQ
                    qsrc = qt_sb if hh == 0 else qtp_sb
                    sp = psB.tile([P, KG, TQ], F32, tag="sp", bufs=2, name="sp")
                    for j in range(KG):
                        k = g * KG + j
                        nc.tensor.matmul(
                            sp[:, j, :],
                            ktp_sb[hh][:, b * T + k * KC: b * T + (k + 1) * KC],
                            qsrc[:, base:base + TQ],
                            start=True, stop=True,
                        )
                    st["sps"][g] = sp

                def attention(i):
                    u = units[i]
                    b, qc, hh = u
                    base = b * T + qc * TQ
                    hs = slice(hh * HD, (hh + 1) * HD)
                    st = unit_state(u)
                    if 0 not in st["sps"]:  # first unit only
                        do_S(u, 0)
                        do_S(u, 1)
                    op_t = st["op_t"]
                    for g in range(NG):
                        if tasks:
                            t = tasks.pop(0)
                            if t is not None:
                                t()
                        if g == 1:
                            flush(pend_a)
                        elif g == 2:
                            flush(pend_b)
                        if g + 2 < NG:
                            do_S(u, g + 2)
                        elif i + 1 < len(units):
                            do_S(units[i + 1], g + 2 - NG)
                        pt = ptp.tile([P, KG, TQ], BF16, tag="pt", bufs=5, name="pt")
                        nc.scalar.activation(pt[:], st["sps"].pop(g)[:], Exp)
                        for j in range(KG):
                            k = g * KG + j
                            nc.tensor.matmul(
                                op_t[:],
                                v_sb[:, b * TB + k, hh * 128: (hh + 1) * 128],
                                pt[:, j, :],
                                start=(g == 0 and j == 0),
                                stop=(g == NG - 1 and j == KG - 1),
                            )
                    del state[u]

                    pe_bcast = i >= len(units) - 2

                    def drain_a(op_t=op_t, pe_bcast=pe_bcast):
                        # 1/denominator broadcast across the 64 head-dim
                        # partitions: gpsimd normally; the last unit pair
                        # uses a K=1 PE matmul instead so the final
                        # allgather isn't queued behind gpsimd work
                        rc0 = stage.tile([1, TQ], F32, tag="rc0", bufs=3, name="rc0")
                        nc.vector.tensor_copy(rc0[:], op_t[HD:HD + 1, :])
                        rc = stage.tile([1, TQ], F32, tag="rc", bufs=3, name="rc")
                        nc.vector.reciprocal_approx_fast(rc[:], rc0[:])
                        bc_sb = stage.tile([HD, TQ], F32, tag="bc_sb", bufs=3, name="bc_sb")
                        if pe_bcast:
                            bc_ps = mm_ps()
                            nc.tensor.matmul(
                                bc_ps[0:HD, :], ones1[0:1, 0:HD], rc[:],
                                start=True, stop=True)
                            nc.vector.tensor_copy(bc_sb[:], bc_ps[0:HD, :])
                        else:
                            nc.gpsimd.partition_broadcast(bc_sb[:], rc[:])

                        def drain_b(op_t=op_t, bc_sb=bc_sb, hs=hs, base=base):
                            nc.vector.tensor_mul(
                                ot_sb[hs, base:base + TQ], op_t[0:HD, :], bc_sb[:])

                        pend_b.append(drain_b)

                    pend_a.append(drain_a)

                def allgather(b, qc):
                    flush_all()
                    base = b * T + qc * TQ
                    nc.sync.dma_start(ag_in[(b, qc)][:], ot_sb[:, base:base + TQ])
                    nc.gpsimd.collective_compute(
                        "AllGather",
                        mybir.AluOpType.bypass,
                        ins=[ag_in[(b, qc)].opt()],
                        outs=[ag_out[(b, qc)].opt()],
                        replica_groups=rg,
                    )

                def proj_load(b, qc, eng=None):
                    # split per core-slice so proj matmul r can start as
                    # soon as slice r lands (instead of after the full 1MB)
                    g = (b * T + qc * TQ) // GRL
                    src = ag_out[(b, qc)].rearrange("(n p) t -> p n t", p=P)
                    for r in range(NCORE):
                        (eng or nc.sync).dma_start(
                            xt_sb[:, g, r, :], src[:, r, :])

                def proj_mm(b, qc):
                    # transposed projection: out^T[outcol, token] so wp is the
                    # stationary operand and tokens stream 512 wide
                    base = b * T + qc * TQ
                    g = base // GRL
                    pp = mm_ps()
                    for r in range(NCORE):
                        nc.tensor.matmul(
                            pp[:], wp_sb[:, r, :], xt_sb[:, g, r, :],
                            start=(r == 0), stop=(r == NCORE - 1),
                        )
                    ost = stage.tile([CSL, TQ], F32, tag="ost", bufs=2, name="ost")
                    nc.vector.tensor_scalar_add(ost[:], pp[:], bp_sb[:])
                    nc.sync.dma_start(out_ext[:, base:base + TQ], ost[:])

                # ---- unit schedule ----
                for qc in range(NQC):
                    attention(qc * 2)
                    attention(qc * 2 + 1)
                    allgather(0, qc)
                assert not tasks, f"{len(tasks)} QKV tasks left after b0"
                # b1: proj(0,qc) injected during (1,qc) - its allgather
                # completed >=2 unit-pairs earlier and its staging load is
                # issued one unit-pair ahead. proj(1,qc) lags 2 unit-pairs.
                proj_load(0, 0)
                for qc in range(NQC):
                    if qc == 0:
                        tasks.extend([None] * 10)
                    tasks.append(lambda qc=qc: proj_mm(0, qc))
                    if qc < NQC - 1:
                        tasks.append(lambda qc=qc: proj_load(0, qc + 1))
                    if qc >= 2:
                        tasks.append(lambda qc=qc: proj_load(1, qc - 2))
                        tasks.extend([None] * 9)
                        tasks.append(lambda qc=qc: proj_mm(1, qc - 2))
                    attention(8 + qc * 2)
                    attention(8 + qc * 2 + 1)
                    allgather(1, qc)
                    assert not [t for t in tasks if t is not None]
                    del tasks[:]
                # tail: the last two staging loads ride the scalar ring --
                # every EXP has already been issued, and parking there
                # cannot block the final ag_in write on the sync ring
                proj_load(1, 2, eng=nc.scalar)
                flush_all()
                proj_mm(1, 2)
                proj_load(1, 3, eng=nc.scalar)
                proj_mm(1, 3)

    nc.compile()
    return nc


def _shard_inputs(x, w_qkv, b_qkv, w_proj, b_proj):
    import ml_dtypes

    bf16 = ml_dtypes.bfloat16
    sc = np.float32(HD ** -0.5)
    # x granule layout [P, NGR, NCC, GRL]: x3[p, g, c, t] = x[g*GRL+t, c*128+p]
    x3 = np.ascontiguousarray(
        x.reshape(NGR, GRL, NCC, P).astype(bf16).transpose(3, 0, 2, 1))

    def wtile(w):
        # [1024, 128] -> [P, NCC, CSL]: wt[p, c, d] = w[c*128+p, d]
        return np.ascontiguousarray(w.reshape(NCC, P, CSL).transpose(1, 0, 2))

    in_maps = []
    for i in range(NCORE):
        h0 = 2 * i
        cs = slice(h0 * HD, h0 * HD + CSL)
        es = slice(i * CSL, (i + 1) * CSL)
        m = {
            "x": x3,
            "wq": wtile((w_qkv[:, 0 * C:1 * C][:, cs] * sc).astype(bf16)),
            "wk": wtile(w_qkv[:, 1 * C:2 * C][:, cs].astype(bf16)),
            "wv": wtile(w_qkv[:, 2 * C:3 * C][:, cs].astype(bf16)),
            "wp": wtile(w_proj[:, es].astype(bf16)),
            "bq": np.ascontiguousarray((b_qkv[0 * C:1 * C][cs] * sc).reshape(CSL, 1), dtype=np.float32),
            "bk": np.ascontiguousarray(b_qkv[1 * C:2 * C][cs].reshape(CSL, 1), dtype=np.float32),
            "bv": np.ascontiguousarray(b_qkv[2 * C:3 * C][cs].reshape(1, CSL), dtype=np.float32),
            "bp": np.ascontiguousarray(b_proj[es].reshape(CSL, 1), dtype=np.float32),
        }
        in_maps.append(m)
    return in_maps


def _run(inputs, trace=False):
    from concourse.bass_utils import run_bass_kernel_spmd

    if "nc" not in _CACHE:
        _CACHE["nc"] = _build()
    nc = _CACHE["nc"]
    in_maps = _shard_inputs(
        np.asarray(inputs["x"]), np.asarray(inputs["w_qkv"]), np.asarray(inputs["b_qkv"]),
        np.asarray(inputs["w_proj"]), np.asarray(inputs["b_proj"]))
    res = run_bass_kernel_spmd(nc, in_maps, list(range(NCORE)), trace=trace)
    # each core returns its [CSL, BT] slice of out^T; stack and transpose
    out = np.concatenate([np.asarray(res.results[i]["out"]) for i in range(NCORE)], axis=0)
    return out.T.reshape(B, T, C).astype(np.float32), res


def kernel(**inputs) -> np.ndarray:
    out, _ = _run(inputs, trace=False)
    return out


# revision 5
# speedup vs baseline: 1.0244x; 1.0244x over previous
"""Distributed attention kernel for one TRN2 chip (8 NeuronCores), v2.

Sharding: 16 heads / 8 cores = 2 heads per core (head-group parallel).

The v2 schedule is built around three hardware facts:
  - the PE clock is HAM-gated: ~1.2 GHz until ~3.4us of continuous matmul
    activity, 2.4 GHz after - so the kernel front-loads dummy matmuls to
    warm the array while x streams in, and keeps the PE queue non-empty
    throughout;
  - DMA descriptor generation serializes on the two HWDGE rings - so x and
    the weights are pre-laid-out on the host such that every DMA is 128
    contiguous 8KB (x granule halves) / 2KB (weights) descriptors, with x
    arriving in 512-token granules split across both rings;
  - the ACT (scalar) sequencer issues nothing but EXP once attention
    starts - all steady-state DMAs (allgather in/out, output writes) sit on
    the sync ring.

Dataflow per core: QKV for its 2 heads; attention per (batch, head,
512-query chunk) with scores in [k, q] layout (P^T feeds the PV matmul
directly), exp on ACT (scores bounded - no max subtraction), softmax
denominator via a ones column in V. The 16 attention units form a single
software pipeline: each unit emits the NEXT unit's first two score
matmuls in its last two groups, so the ACT engine never waits at unit or
allgather boundaries. Remaining QKV chunks and the output projections are
an ordered task stream injected one per group. AllGather of bf16
c-slices per (batch, qc) runs under later units; projection against the
core's 128-column w_proj slice is pipelined behind its allgather. Host
concatenates the 8 column slices.
"""

import numpy as np

_CACHE = {}

P = 128
B, T, C = 2, 2048, 1024
BT = B * T
NCORE = 8
HD = 64  # head dim
CSL = 128  # per-core c-slice = 2 heads * 64
TQ = 512  # query chunk
NQC = T // TQ  # 4
KC = 128  # key chunk (partition dim)
NKC = T // KC  # 16
KG = 2  # key chunks per exp group
NG = NKC // KG  # 8
NCC = C // P  # 8 contraction chunks
NTC = BT // P  # 32 token chunks of 128
TB = T // P  # 16 token chunks per batch
NGR = 8  # x arrival granules (512 tokens each)
GRL = BT // NGR  # 512


def _build():
    import concourse.bass as bass
    import concourse.tile as tile
    from concourse import bacc, mybir

    F32 = mybir.dt.float32
    BF16 = mybir.dt.bfloat16
    Exp = mybir.ActivationFunctionType.Exp

    nc = bacc.Bacc("TRN2", target_bir_lowering=False, debug=False, num_devices=NCORE)

    # host-prepared layouts (see _shard_inputs):
    #   x:  [P, NGR, NCC, GRL]  (c-chunk-major, 512-token granules)
    #   w*: [P, NCC, CSL]
    x_ext = nc.declare_dram_parameter("x", [P, NGR, NCC, GRL], BF16, isOutput=False)
    wq_ext = nc.declare_dram_parameter("wq", [P, NCC, CSL], BF16, isOutput=False)
    wk_ext = nc.declare_dram_parameter("wk", [P, NCC, CSL], BF16, isOutput=False)
    wv_ext = nc.declare_dram_parameter("wv", [P, NCC, CSL], BF16, isOutput=False)
    wp_ext = nc.declare_dram_parameter("wp", [P, NCC, CSL], BF16, isOutput=False)
    bq_ext = nc.declare_dram_parameter("bq", [CSL, 1], F32, isOutput=False)
    bk_ext = nc.declare_dram_parameter("bk", [CSL, 1], F32, isOutput=False)
    bv_ext = nc.declare_dram_parameter("bv", [1, CSL], F32, isOutput=False)
    bp_ext = nc.declare_dram_parameter("bp", [CSL, 1], F32, isOutput=False)
    # transposed output [CSL, BT]; the host transposes back
    out_ext = nc.declare_dram_parameter("out", [CSL, BT], F32, isOutput=True)

    rg = [list(range(NCORE))]

    with tile.TileContext(nc) as tc:
        with (
            nc.allow_low_precision("bf16 attention compute by design"),
            tc.tile_pool(name="pers", bufs=1) as pers,
            tc.tile_pool(name="stage", bufs=3) as stage,
            tc.tile_pool(name="dram", bufs=1, space="DRAM") as dram,
        ):
            # ---- persistent SBUF tiles ----
            xt_sb = pers.tile([P, NGR, NCC, GRL], BF16, name="xt_sb")
            qt_sb = pers.tile([P, BT], BF16, name="qt_sb")  # Q^T (rows: 2*64 head dims)
            kt_sb = pers.tile([P, BT], BF16, name="kt_sb")
            # per-head score operands with the head's 64 dims at partitions
            # 0-63 and zeros above: K=128 stationary tiles keep fast weight
            # load enabled for the score matmuls
            ktp_sb = [pers.tile([P, BT], BF16, name=f"ktp{h}_sb") for h in range(2)]
            qtp_sb = pers.tile([P, BT], BF16, name="qtp_sb")  # h1 only
            v_sb = pers.tile([P, NTC, 256], BF16, name="v_sb")  # V + ones col, 128/head
            ot_sb = pers.tile([P, BT], BF16, name="ot_sb")  # attention out^T
            wq_sb = pers.tile([P, NCC, CSL], BF16, name="wq_sb")
            wk_sb = pers.tile([P, NCC, CSL], BF16, name="wk_sb")
            wv_sb = pers.tile([P, NCC, CSL], BF16, name="wv_sb")
            wp_sb = pers.tile([P, NCC, CSL], BF16, name="wp_sb")
            bq_sb = pers.tile([CSL, 1], F32, name="bq_sb")
            bk_sb = pers.tile([CSL, 1], F32, name="bk_sb")
            bv_row = pers.tile([1, CSL], F32, name="bv_row")
            bp_sb = pers.tile([CSL, 1], F32, name="bp_sb")
            bv_bc = pers.tile([P, CSL], F32, name="bv_bc")
            ones1 = pers.tile([1, P], F32, name="ones1")
            warm_sb = pers.tile([P, 640], BF16, name="warm_sb")
            act_seed = pers.tile([1, 2], F32, name="act_seed")

            nc.gpsimd.memset(ones1[:], 1.0)
            nc.gpsimd.memset(warm_sb[:], 0.0)
            nc.gpsimd.memset(act_seed[:], 0.0)
            barrier_cc = []  # emitted onto the gpsimd queue after bar_in's DMA
            nc.gpsimd.memset(v_sb[:, :, 65:128], 0.0)
            nc.gpsimd.memset(v_sb[:, :, 193:256], 0.0)
            nc.gpsimd.memset(v_sb[:, :, 64], 1.0)
            nc.gpsimd.memset(v_sb[:, :, 192], 1.0)
            nc.gpsimd.memset(ktp_sb[0][HD:P, :], 0.0)
            nc.gpsimd.memset(ktp_sb[1][HD:P, :], 0.0)
            nc.gpsimd.memset(qtp_sb[:], 0.0)

            # ---- DMA issue: fast small stuff first, then x granules with
            # halves split across the two HWDGE rings so granule i lands
            # strictly before granule i+1.
            nc.sync.dma_start(bq_sb[:], bq_ext[:])
            nc.sync.dma_start(bk_sb[:], bk_ext[:])
            nc.sync.dma_start(bv_row[:], bv_ext[:])
            nc.sync.dma_start(bp_sb[:], bp_ext[:])
            for ext, dst in ((wk_ext, wk_sb), (wv_ext, wv_sb), (wq_ext, wq_sb), (wp_ext, wp_sb)):
                nc.scalar.dma_start(dst[:], ext[:])
            for g in range(NGR):
                nc.sync.dma_start(xt_sb[:, g, 0:4, :], x_ext[:, g, 0:4, :])
                nc.scalar.dma_start(xt_sb[:, g, 4:8, :], x_ext[:, g, 4:8, :])

            # preload the ACT Exp table while DMA streams
            nc.scalar.activation(act_seed[0:1, 0:1], act_seed[0:1, 1:2], Exp)

            # tiny barrier collective at t0: absorbs the cross-core start
            # skew while the PE warms up, so the first real allgather's
            # barrier doesn't eat it
            bar_in = dram.tile([1, 2], F32, tag="bar_in", name="bar_in")
            bar_out = dram.tile([NCORE, 2], F32, addr_space="Shared",
                                tag="bar_out", name="bar_out")
            nc.sync.dma_start(bar_in[:], act_seed[:])
            nc.gpsimd.collective_compute(
                "AllGather",
                mybir.AluOpType.bypass,
                ins=[bar_in.opt()],
                outs=[bar_out.opt()],
                replica_groups=rg,
            )

            def xt(c, t0, n):
                g, off = divmod(t0, GRL)
                assert off + n <= GRL
                return xt_sb[:, g, c, off:off + n]

            ag_in = {}
            ag_out = {}
            for b in range(B):
                for qc in range(NQC):
                    ag_in[(b, qc)] = dram.tile(
                        [CSL, TQ], BF16, tag=f"agin_{b}_{qc}", name=f"agin_{b}_{qc}")
                    ag_out[(b, qc)] = dram.tile(
                        [NCORE * CSL, TQ], BF16, addr_space="Shared",
                        tag=f"agout_{b}_{qc}", name=f"agout_{b}_{qc}")

            with (
                tc.tile_pool(name="psB", bufs=1, space="PSUM") as psB,
                tc.tile_pool(name="ptp", bufs=3) as ptp,
            ):
                def mm_ps():
                    return psB.tile([P, TQ], F32, tag="mm", bufs=2, name="ps_mm")

                def qkv_qk_chunk(w_sb, b_sb, dst, t8):
                    ps = mm_ps()
                    rng = slice(t8 * TQ, (t8 + 1) * TQ)
                    for c in range(NCC):
                        nc.tensor.matmul(
                            ps[:], w_sb[:, c, :], xt(c, t8 * TQ, TQ),
                            start=(c == 0), stop=(c == NCC - 1),
                        )
                    if dst is kt_sb:
                        # h0 rows straight into the padded tile; h1 rows
                        # staged then partition-shifted down via DMA
                        nc.vector.tensor_scalar_add(
                            ktp_sb[0][0:HD, rng], ps[0:HD, :], b_sb[0:HD, :])
                        nc.vector.tensor_scalar_add(
                            kt_sb[HD:P, rng], ps[HD:P, :], b_sb[HD:P, :])
                        nc.sync.dma_start(ktp_sb[1][0:HD, rng], kt_sb[HD:P, rng])
                    else:
                        nc.vector.tensor_scalar_add(qt_sb[:, rng], ps[:], b_sb[:])
                        nc.sync.dma_start(qtp_sb[0:HD, rng], qt_sb[HD:P, rng])

                def qkv_v_pair(i):
                    # two 128-token V chunks (half an x granule)
                    for j in (i, i + 1):
                        ps = mm_ps()
                        for c in range(NCC):
                            nc.tensor.matmul(
                                ps[:, 0:CSL], xt(c, j * P, P), wv_sb[:, c, :],
                                start=(c == 0), stop=(c == NCC - 1),
                            )
                        nc.vector.tensor_add(v_sb[:, j, 0:HD], ps[:, 0:HD], bv_bc[:, 0:HD])
                        nc.vector.tensor_add(
                            v_sb[:, j, 128:128 + HD], ps[:, HD:2 * HD], bv_bc[:, HD:2 * HD])

                def warmup(n):
                    # dead-write matmuls that flip the PE HAM to full clock
                    # while x streams in
                    for _ in range(n):
                        ps = mm_ps()
                        nc.tensor.matmul(
                            ps[:], warm_sb[:, 0:128], warm_sb[:, 128:640],
                            start=True, stop=True)

                # broadcast the V free-axis bias across partitions (K=1 matmul)
                bb = mm_ps()
                nc.tensor.matmul(bb[:, 0:CSL], ones1[0:1, :], bv_row[:], start=True, stop=True)
                nc.vector.tensor_copy(bv_bc[:], bb[:, 0:CSL])

                warmup(7)

                # minimal pre-attention QKV: K t8=0,1 / V chunks 0-3 / Q t8=0
                qkv_qk_chunk(wk_sb, bk_sb, kt_sb, 0)
                qkv_qk_chunk(wk_sb, bk_sb, kt_sb, 1)
                qkv_v_pair(0)
                qkv_v_pair(2)
                qkv_qk_chunk(wq_sb, bq_sb, qt_sb, 0)

                # remaining QKV as an ordered task stream injected one per
                # group; order tracks x-granule arrival and first-use
                # deadlines inside unit (0,0,h0).
                tasks = [
                    lambda: qkv_qk_chunk(wk_sb, bk_sb, kt_sb, 2),
                    lambda: qkv_v_pair(4),
                    lambda: qkv_v_pair(6),
                    lambda: qkv_qk_chunk(wk_sb, bk_sb, kt_sb, 3),
                    lambda: qkv_v_pair(8),
                    lambda: qkv_v_pair(10),
                    lambda: qkv_v_pair(12),
                    lambda: qkv_v_pair(14),
                    lambda: qkv_qk_chunk(wq_sb, bq_sb, qt_sb, 1),
                    lambda: qkv_qk_chunk(wk_sb, bk_sb, kt_sb, 4),
                    lambda: qkv_qk_chunk(wk_sb, bk_sb, kt_sb, 5),
                    lambda: qkv_qk_chunk(wq_sb, bq_sb, qt_sb, 2),
                    lambda: qkv_qk_chunk(wk_sb, bk_sb, kt_sb, 6),
                    lambda: qkv_qk_chunk(wk_sb, bk_sb, kt_sb, 7),
                    lambda: qkv_qk_chunk(wq_sb, bq_sb, qt_sb, 3),
                    lambda: qkv_v_pair(16),
                    lambda: qkv_v_pair(18),
                    lambda: qkv_v_pair(20),
                    lambda: qkv_v_pair(22),
                    lambda: qkv_v_pair(24),
                    lambda: qkv_v_pair(26),
                    lambda: qkv_v_pair(28),
                    lambda: qkv_v_pair(30),
                    lambda: qkv_qk_chunk(wq_sb, bq_sb, qt_sb, 4),
                    lambda: qkv_qk_chunk(wq_sb, bq_sb, qt_sb, 5),
                    lambda: qkv_qk_chunk(wq_sb, bq_sb, qt_sb, 6),
                    lambda: qkv_qk_chunk(wq_sb, bq_sb, qt_sb, 7),
                ]

                pend_a = []  # drain part A: denom copy + reciprocal + bcast
                pend_b = []  # drain part B: DVE normalize-multiply

                def flush(which):
                    while which:
                        which.pop(0)()

                def flush_all():
                    flush(pend_a)
                    flush(pend_b)

                # ---- attention unit pipeline ----
                units = [(b, qc, hh) for b in range(B) for qc in range(NQC)
                         for hh in range(2)]
                state = {}

                def unit_state(u):
                    if u not in state:
                        state[u] = {
                            "op_t": psB.tile([P, TQ], F32, tag="acc", bufs=2, name="op_t"),
                            "sps": {},
                        }
                    return state[u]

                def do_S(u, g):
                    b, qc, hh = u
                    st = unit_state(u)
                    base = b * T + qc * TQ
                    qsrc = qt_sb if hh == 0 else qtp_sb
                    sp = psB.tile([P, KG, TQ], F32, tag="sp", bufs=2, name="sp")
                    for j in range(KG):
                        k = g * KG + j
                        nc.tensor.matmul(
                            sp[:, j, :],
                            ktp_sb[hh][:, b * T + k * KC: b * T + (k + 1) * KC],
                            qsrc[:, base:base + TQ],
                            start=True, stop=True,
                        )
                    st["sps"][g] = sp

                def attention(i):
                    u = units[i]
                    b, qc, hh = u
                    base = b * T + qc * TQ
                    hs = slice(hh * HD, (hh + 1) * HD)
                    st = unit_state(u)
                    if 0 not in st["sps"]:  # first unit only
                        do_S(u, 0)
                        do_S(u, 1)
                    op_t = st["op_t"]
                    for g in range(NG):
                        if tasks:
                            t = tasks.pop(0)
                            if t is not None:
                                t()
                        if g == 1:
                            flush(pend_a)
                        elif g == 2:
                            flush(pend_b)
                        if g + 2 < NG:
                            do_S(u, g + 2)
                        elif i + 1 < len(units):
                            do_S(units[i + 1], g + 2 - NG)
                        pt = ptp.tile([P, KG, TQ], BF16, tag="pt", bufs=5, name="pt")
                        nc.scalar.activation(pt[:], st["sps"].pop(g)[:], Exp)
                        for j in range(KG):
                            k = g * KG + j
                            nc.tensor.matmul(
                                op_t[:],
                                v_sb[:, b * TB + k, hh * 128: (hh + 1) * 128],
                                pt[:, j, :],
                                start=(g == 0 and j == 0),
                                stop=(g == NG - 1 and j == KG - 1),
                            )
                    del state[u]

                    pe_bcast = i >= len(units) - 2

                    def drain_a(op_t=op_t, pe_bcast=pe_bcast):
                        # 1/denominator broadcast across the 64 head-dim
                        # partitions: gpsimd normally; the last unit pair
                        # uses a K=1 PE matmul instead so the final
                        # allgather isn't queued behind gpsimd work
                        rc0 = stage.tile([1, TQ], F32, tag="rc0", bufs=3, name="rc0")
                        nc.vector.tensor_copy(rc0[:], op_t[HD:HD + 1, :])
                        rc = stage.tile([1, TQ], F32, tag="rc", bufs=3, name="rc")
                        nc.vector.reciprocal_approx_fast(rc[:], rc0[:])
                        bc_sb = stage.tile([HD, TQ], F32, tag="bc_sb", bufs=3, name="bc_sb")
                        if pe_bcast:
                            bc_ps = mm_ps()
                            nc.tensor.matmul(
                                bc_ps[0:HD, :], ones1[0:1, 0:HD], rc[:],
                                start=True, stop=True)
                            nc.vector.tensor_copy(bc_sb[:], bc_ps[0:HD, :])
                        else:
                            nc.gpsimd.partition_broadcast(bc_sb[:], rc[:])

                        def drain_b(op_t=op_t, bc_sb=bc_sb, hs=hs, base=base):
                            nc.vector.tensor_mul(
                                ot_sb[hs, base:base + TQ], op_t[0:HD, :], bc_sb[:])

                        pend_b.append(drain_b)

                    pend_a.append(drain_a)

                def allgather(b, qc):
                    flush_all()
                    base = b * T + qc * TQ
                    nc.sync.dma_start(ag_in[(b, qc)][:], ot_sb[:, base:base + TQ])
                    nc.gpsimd.collective_compute(
                        "AllGather",
                        mybir.AluOpType.bypass,
                        ins=[ag_in[(b, qc)].opt()],
                        outs=[ag_out[(b, qc)].opt()],
                        replica_groups=rg,
                    )

                def proj_load(b, qc, eng=None):
                    # split per core-slice so proj matmul r can start as
                    # soon as slice r lands (instead of after the full 1MB)
                    g = (b * T + qc * TQ) // GRL
                    src = ag_out[(b, qc)].rearrange("(n p) t -> p n t", p=P)
                    for r in range(NCORE):
                        (eng or nc.sync).dma_start(
                            xt_sb[:, g, r, :], src[:, r, :])

                # Tile's scheduler reorders by simulated readiness and
                # underestimates when each allgather really completes on HW;
                # pin each projection's scheduling timestamp past its
                # collective's measured completion so the proj matmuls don't
                # get placed where they head-of-line-block the PE.
                proj_wait_ms = {
                    (0, 0): 0.122, (0, 1): 0.152, (0, 2): 0.176, (0, 3): 0.198,
                    (1, 0): 0.222, (1, 1): 0.244, (1, 2): 0.266, (1, 3): 0.284,
                }

                def proj_mm(b, qc):
                    # transposed projection: out^T[outcol, token] so wp is the
                    # stationary operand and tokens stream 512 wide
                    base = b * T + qc * TQ
                    g = base // GRL
                    with tc.tile_wait_until(proj_wait_ms[(b, qc)]):
                        pp = mm_ps()
                        for r in range(NCORE):
                            nc.tensor.matmul(
                                pp[:], wp_sb[:, r, :], xt_sb[:, g, r, :],
                                start=(r == 0), stop=(r == NCORE - 1),
                            )
                        ost = stage.tile([CSL, TQ], F32, tag="ost", bufs=2, name="ost")
                        nc.vector.tensor_scalar_add(ost[:], pp[:], bp_sb[:])
                        nc.sync.dma_start(out_ext[:, base:base + TQ], ost[:])

                # ---- unit schedule ----
                for qc in range(NQC):
                    attention(qc * 2)
                    attention(qc * 2 + 1)
                    allgather(0, qc)
                assert not tasks, f"{len(tasks)} QKV tasks left after b0"
                # b1: proj(0,qc) injected during (1,qc) - its allgather
                # completed >=2 unit-pairs earlier and its staging load is
                # issued one unit-pair ahead. proj(1,qc) lags 2 unit-pairs.
                proj_load(0, 0)
                for qc in range(NQC):
                    if qc == 0:
                        tasks.extend([None] * 10)
                    tasks.append(lambda qc=qc: proj_mm(0, qc))
                    if qc < NQC - 1:
                        tasks.append(lambda qc=qc: proj_load(0, qc + 1))
                    if qc >= 2:
                        tasks.append(lambda qc=qc: proj_load(1, qc - 2))
                        tasks.extend([None] * 9)
                        tasks.append(lambda qc=qc: proj_mm(1, qc - 2))
                    attention(8 + qc * 2)
                    attention(8 + qc * 2 + 1)
                    allgather(1, qc)
                    assert not [t for t in tasks if t is not None]
                    del tasks[:]
                # tail: the last two staging loads ride the scalar ring --
                # every EXP has already been issued, and parking there
                # cannot block the final ag_in write on the sync ring
                proj_load(1, 2, eng=nc.scalar)
                flush_all()
                proj_mm(1, 2)
                proj_load(1, 3, eng=nc.scalar)
                proj_mm(1, 3)

    nc.compile()
    return nc


def _shard_inputs(x, w_qkv, b_qkv, w_proj, b_proj):
    import ml_dtypes

    bf16 = ml_dtypes.bfloat16
    sc = np.float32(HD ** -0.5)
    # x granule layout [P, NGR, NCC, GRL]: x3[p, g, c, t] = x[g*GRL+t, c*128+p]
    x3 = np.ascontiguousarray(
        x.reshape(NGR, GRL, NCC, P).astype(bf16).transpose(3, 0, 2, 1))

    def wtile(w):
        # [1024, 128] -> [P, NCC, CSL]: wt[p, c, d] = w[c*128+p, d]
        return np.ascontiguousarray(w.reshape(NCC, P, CSL).transpose(1, 0, 2))

    in_maps = []
    for i in range(NCORE):
        h0 = 2 * i
        cs = slice(h0 * HD, h0 * HD + CSL)
        es = slice(i * CSL, (i + 1) * CSL)
        m = {
            "x": x3,
            "wq": wtile((w_qkv[:, 0 * C:1 * C][:, cs] * sc).astype(bf16)),
            "wk": wtile(w_qkv[:, 1 * C:2 * C][:, cs].astype(bf16)),
            "wv": wtile(w_qkv[:, 2 * C:3 * C][:, cs].astype(bf16)),
            "wp": wtile(w_proj[:, es].astype(bf16)),
            "bq": np.ascontiguousarray((b_qkv[0 * C:1 * C][cs] * sc).reshape(CSL, 1), dtype=np.float32),
            "bk": np.ascontiguousarray(b_qkv[1 * C:2 * C][cs].reshape(CSL, 1), dtype=np.float32),
            "bv": np.ascontiguousarray(b_qkv[2 * C:3 * C][cs].reshape(1, CSL), dtype=np.float32),
            "bp": np.ascontiguousarray(b_proj[es].reshape(CSL, 1), dtype=np.float32),
        }
        in_maps.append(m)
    return in_maps


def _run(inputs, trace=False):
    from concourse.bass_utils import run_bass_kernel_spmd

    if "nc" not in _CACHE:
        _CACHE["nc"] = _build()
    nc = _CACHE["nc"]
    in_maps = _shard_inputs(
        np.asarray(inputs["x"]), np.asarray(inputs["w_qkv"]), np.asarray(inputs["b_qkv"]),
        np.asarray(inputs["w_proj"]), np.asarray(inputs["b_proj"]))
    res = run_bass_kernel_spmd(nc, in_maps, list(range(NCORE)), trace=trace)
    # each core returns its [CSL, BT] slice of out^T; stack and transpose
    out = np.concatenate([np.asarray(res.results[i]["out"]) for i in range(NCORE)], axis=0)
    return out.T.reshape(B, T, C).astype(np.float32), res


def kernel(**inputs) -> np.ndarray:
    out, _ = _run(inputs, trace=False)
    return out
